# revision 28
# baseline (speedup 1.0000x reference)
"""GPTQ 4-bit quantized linear layer on 8 TRN2 NeuronCores.

Problem: x [4, 2048, 4096] f32, packed_weight [4096, 2048] int32 (two uint4
per byte), scales/zeros [4096, 64] f32, bias [4096] f32.
out = x @ dequant(W).T + bias, out [4, 2048, 4096] f32.

Strategy
--------
- Shard M = 8192 rows of x across the 8 cores (data parallel, 1024 rows
  each); replicate the (small) weight-side tensors. Per-core HBM traffic
  (~56 MiB) is far below PE time, so the kernel is TensorE-bound at the
  bf16 matmul roofline (~437 us/core of pure streaming).
- k-permutation: contraction order k' = [all even k | all odd k] applied
  consistently to x and W, so nibble unpacking needs no interleave. The
  k'-tile index is kt = t + 16h (t = byte-column tile, h = nibble).
- The host pre-arranges everything into the exact SBUF images the
  matmul wants, so the device performs NO transposes at all (device
  DMA-transposes were measured to serialize the whole DMA subsystem on
  every DMACopy<->DMATranspose mode flip):
    * x: bf16, [m-tile][partition(k'), k'-tile, m] - plain 1 MiB DMAs.
    * q: unpacked nibbles as uint8 in [chunk][partition(k'), t, h, n]
    * s: group scales expanded to [chunk][partition(k'), t, n] bf16
      (rows 32r..32r+31 of tile t hold s[n, 4t+r]; shared by both
      nibble halves of byte-tile t).
- On-device dequant is one fused DVE op per (chunk, t):
  wt[:, t, h, n] = (q_img - 7.5) * s_img (s broadcast over h via a
  step-0 AP), writing straight into the double-buffered weight slab.
  Centering q halves |w| and hence its bf16 rounding error.
- zeros/bias are folded into a rank-65 bf16 matmul accumulated into the
  same PSUM tiles: out += A.T @ C with A[g, m] = group sums of bf16(x)
  plus a ones row (host-computed), C[g, n] = ((7.5 - z)*s).T plus the
  bias row. Using bf16(x) for A makes the x-rounding error cancel
  group-wise against the z-part of the weight.

Measured on trn2.8x1: ~492-497 us HW exec (max over the 8 cores),
relative error ~2.6e-3, TensorE ~86% MFU with the matmul stream pacing
at the 216 ns/MM bf16 roofline.
"""

import numpy as np
import ml_dtypes

import concourse.tile as tile
from concourse import bacc, mybir
from concourse import bass_utils

P = 128
K = 4096
K2 = K // 2
N = 4096
G = 64               # number of groups (K // 64)
GROUPSIZE = 64
N_CORES = 8
M_TOTAL = 8192
M_C = M_TOTAL // N_CORES   # rows per core

NT_CHUNK = 512       # matmul free dim (n per PSUM tile)
KT = K // P          # 32 k'-tiles
TT = KT // 2         # 16 byte-column tiles (nibble halves share scales)
NTC = N // NT_CHUNK  # n-chunks

F32 = mybir.dt.float32
BF16 = mybir.dt.bfloat16
U8 = mybir.dt.uint8

BF16_NP = np.dtype(ml_dtypes.bfloat16)


def build_nc(m_c=M_C, n=N, num_devices=N_CORES, corr_dt=mybir.dt.bfloat16):
    """Build the per-core Bass program (SPMD, no collectives)."""
    mt = m_c // P
    ntc = n // NT_CHUNK

    nc = bacc.Bacc("TRN2", target_bir_lowering=False, debug=False,
                   num_devices=num_devices)

    # x image: [m-tile, partition, k'-tile, m] bf16
    x_d = nc.dram_tensor("x", [mt, P, KT, P], BF16,
                         kind="ExternalInput").ap()
    # q image: [chunk, partition, t, half, n-slice] uint8
    q_d = nc.dram_tensor("q", [ntc, P, TT, 2, NT_CHUNK], U8,
                         kind="ExternalInput").ap()
    # scale image: [chunk, partition, t, n-slice] fp16 (10-bit mantissa
    # keeps the scale rounding negligible; s in [0.001, 0.021])
    s_d = nc.dram_tensor("s", [ntc, P, TT, NT_CHUNK], mybir.dt.float16,
                         kind="ExternalInput").ap()
    # correction rows 0..63 = -(z*s).T, row 64 = bias  -> [65, n]
    c_d = nc.dram_tensor("c", [G + 1, n], corr_dt, kind="ExternalInput").ap()
    # A rows 0..63 = bf16(x) group sums (transposed), row 64 = ones
    a_d = nc.dram_tensor("a", [G + 1, m_c], corr_dt,
                         kind="ExternalInput").ap()
    out_d = nc.dram_tensor("out", [m_c, n], F32, kind="ExternalOutput").ap()

    with tile.TileContext(nc) as tc:
        with (
            tc.tile_pool(name="const", bufs=1) as constp,
            tc.tile_pool(name="xtp", bufs=1) as xtp_pool,
            tc.tile_pool(name="qim", bufs=2) as qpool,
            tc.tile_pool(name="sim", bufs=2) as spool,
            tc.tile_pool(name="wt", bufs=2) as wtp,
            tc.tile_pool(name="cs", bufs=2) as cpool,
            tc.tile_pool(name="outs", bufs=2) as outp,
            tc.tile_pool(name="ps_out", bufs=6, space="PSUM") as ps_out,
            tc.tile_pool(name="ps_warm", bufs=1, space="PSUM") as ps_warm,
        ):
            a_sb = constp.tile([G + 1, m_c], corr_dt)

            xtp = xtp_pool.tile([P, mt, KT, P], BF16)

            def prep_x(mi):
                # x goes on the Activation HWDGE queue so the startup
                # loads run in parallel with the q/s loads on sync
                nc.scalar.dma_start(xtp[:, mi], x_d[mi])

            wts = [None] * ntc
            css = [None] * ntc

            def emit_chunk_inputs(ci):
                """Loads + dequant multiplies for one 512-wide n-chunk."""
                cs = cpool.tile([G + 1, NT_CHUNK], corr_dt, tag="cs",
                                name="cs")
                nc.sync.dma_start(
                    cs[:], c_d[:, ci * NT_CHUNK:(ci + 1) * NT_CHUNK])
                css[ci] = cs
                qim = qpool.tile([P, TT, 2, NT_CHUNK], U8, tag="qim",
                                 name="qim")
                sim = spool.tile([P, TT, NT_CHUNK], mybir.dt.float16,
                                 tag="sim", name="sim")
                # load in 4-t slices so the first dequant multiply (and
                # hence the first matmul of the chunk) starts ~4x earlier
                for tq in range(0, TT, 4):
                    nc.sync.dma_start(qim[:, tq:tq + 4], q_d[ci, :, tq:tq + 4])
                    nc.sync.dma_start(sim[:, tq:tq + 4], s_d[ci, :, tq:tq + 4])
                wt = wtp.tile([P, TT, 2, NT_CHUNK], BF16, tag="wt",
                              name="wt")
                wts[ci] = wt
                for t in range(TT):
                    s_bc = sim[:, t, None, :].broadcast_to((P, 2, NT_CHUNK))
                    # centered dequant: w = (q - 7.5) * s. Halving the
                    # weight magnitude halves its bf16 rounding error; the
                    # 7.5*s mean moves into the rank-65 correction.
                    nc.vector.scalar_tensor_tensor(
                        out=wt[:, t],
                        in0=qim[:, t],
                        scalar=-7.5,
                        in1=s_bc,
                        op0=mybir.AluOpType.add,
                        op1=mybir.AluOpType.mult,
                    )

            # ---- main loop over n-chunks ----
            prep_x(0)
            emit_chunk_inputs(0)
            nc.sync.dma_start(a_sb[:], a_d[:])
            # HAM warmup: ~7us of dummy matmuls chained on the first cs
            # load so they run during the input DMAs and the PE enters the
            # main loop already at K=8/8 (2.4 GHz)
            wps = ps_warm.tile([P, NT_CHUNK], F32, name="wps")
            for _ in range(16):
                nc.tensor.matmul(
                    wps[:], css[0][:, 0:P], css[0][:], start=True, stop=True)
            for mi in range(1, mt):
                prep_x(mi)

            for ntc_i in range(ntc):
                wt = wts[ntc_i]
                for mi in range(mt):
                    pso = ps_out.tile([P, NT_CHUNK], F32, name="pso")
                    for kt_i in range(KT):
                        # k'-tile kt_i = t + 16h lives at wt[:, t, h, :]
                        h, t = divmod(kt_i, TT)
                        nc.tensor.matmul(
                            pso[:],
                            xtp[:, mi, kt_i],
                            wt[:, t, h],
                            start=(kt_i == 0),
                            stop=False,
                        )
                    # zeros/bias correction: out += A.T @ C
                    nc.tensor.matmul(
                        pso[:],
                        a_sb[:, mi * P:(mi + 1) * P],
                        css[ntc_i][:],
                        start=False,
                        stop=True,
                    )
                    if ntc_i + 1 < ntc and mi == 0:
                        emit_chunk_inputs(ntc_i + 1)
                    ot = outp.tile([P, NT_CHUNK], F32, name="ot")
                    nc.scalar.copy(ot[:], pso[:])
                    nc.scalar.dma_start(
                        out_d[mi * P:(mi + 1) * P,
                              ntc_i * NT_CHUNK:(ntc_i + 1) * NT_CHUNK],
                        ot[:],
                    )

    nc.compile()
    return nc


def prep_inputs(x, packed_weight, scales, zeros, bias):
    """Host-side input preparation -> per-core input maps."""
    xf = x.reshape(M_TOTAL, K)
    # bf16, k'-permuted (even | odd), transposed, tiled to the SBUF image
    xt_bf = np.empty((K, M_TOTAL), dtype=BF16_NP)
    xt_bf[:K2] = xf[:, 0::2].astype(BF16_NP).T
    xt_bf[K2:] = xf[:, 1::2].astype(BF16_NP).T

    pwu = packed_weight.astype(np.uint8)            # values are 0..255
    # q image: [chunk, p, t, half, n-slice]; k' = (t + 16h)*128 + p maps to
    # byte column t*128+p, low nibble for h=0, high nibble for h=1
    b = pwu.T.reshape(TT, P, N)                      # [t, p, n]
    q_img = np.stack([b & 15, b >> 4], axis=2)       # [t, p, 2, n]
    q_img = np.ascontiguousarray(
        q_img.reshape(TT, P, 2, NTC, NT_CHUNK)
        .transpose(3, 1, 0, 2, 4))                   # [chunk, p, t, 2, ns]

    # s image: [chunk, p, t, n-slice] with s_img[p, t, n] = s[n, 4t + p//32]
    sT = scales.astype(np.float32).T                 # [64, N]
    s_img = np.repeat(sT.reshape(TT, 4, 1, N), 32, axis=2)  # [t, 4, 32, n]
    s_img = np.ascontiguousarray(
        s_img.reshape(TT, P, NTC, NT_CHUNK)
        .transpose(2, 1, 0, 3)).astype(np.float16)   # [chunk, p, t, ns]

    c_host = np.concatenate(
        [((7.5 - zeros) * scales).T, bias[None, :]],
        axis=0).astype(BF16_NP)
    # A: per-group sums of bf16(x) plus ones row (bf16 so the x-rounding
    # error cancels against the z-part of the dequantized weight)
    a_full = xf.astype(BF16_NP).astype(np.float32).reshape(
        M_TOTAL, G, GROUPSIZE).sum(axis=2)           # [M, 64]

    in_maps = []
    for c in range(N_CORES):
        sl = slice(c * M_C, (c + 1) * M_C)
        x_img = np.ascontiguousarray(
            xt_bf[:, sl].reshape(KT, P, M_C // P, P).transpose(2, 1, 0, 3))
        a_slab = np.concatenate(
            [a_full[sl].T, np.ones((1, M_C), np.float32)],
            axis=0).astype(BF16_NP)
        in_maps.append({
            "x": x_img,
            "q": q_img,
            "s": s_img,
            "c": c_host,
            "a": np.ascontiguousarray(a_slab),
        })
    return in_maps


_NC_CACHE = {}


def get_nc():
    if "nc" not in _NC_CACHE:
        _NC_CACHE["nc"] = build_nc()
    return _NC_CACHE["nc"]


def kernel(x, packed_weight, scales, zeros, bias):
    x = np.asarray(x)
    packed_weight = np.asarray(packed_weight)
    scales = np.asarray(scales, dtype=np.float32)
    zeros = np.asarray(zeros, dtype=np.float32)
    bias = np.asarray(bias, dtype=np.float32)
    nc = get_nc()
    in_maps = prep_inputs(x, packed_weight, scales, zeros, bias)
    res = bass_utils.run_bass_kernel_spmd(
        nc, in_maps, core_ids=list(range(N_CORES)))
    out = np.concatenate([r["out"] for r in res.results], axis=0)
    return out.reshape(*x.shape[:-1], N).astype(np.float32)



# revision 29
# speedup vs baseline: 1.0062x; 1.0062x over previous
"""GPTQ 4-bit quantized linear layer on 8 TRN2 NeuronCores.

Problem: x [4, 2048, 4096] f32, packed_weight [4096, 2048] int32 (two uint4
per byte), scales/zeros [4096, 64] f32, bias [4096] f32.
out = x @ dequant(W).T + bias, out [4, 2048, 4096] f32.

Strategy
--------
- Shard M = 8192 rows of x across the 8 cores (data parallel, 1024 rows
  each); replicate the (small) weight-side tensors. Per-core HBM traffic
  (~56 MiB) is far below PE time, so the kernel is TensorE-bound at the
  bf16 matmul roofline (~437 us/core of pure streaming).
- k-permutation: contraction order k' = [all even k | all odd k] applied
  consistently to x and W, so nibble unpacking needs no interleave. The
  k'-tile index is kt = t + 16h (t = byte-column tile, h = nibble).
- The host pre-arranges everything into the exact SBUF images the
  matmul wants, so the device performs NO transposes at all (device
  DMA-transposes were measured to serialize the whole DMA subsystem on
  every DMACopy<->DMATranspose mode flip):
    * x: bf16, [m-tile][partition(k'), k'-tile, m] - plain 1 MiB DMAs.
    * q: unpacked nibbles as uint8 in [chunk][partition(k'), t, h, n]
    * s: group scales expanded to [chunk][partition(k'), t, n] bf16
      (rows 32r..32r+31 of tile t hold s[n, 4t+r]; shared by both
      nibble halves of byte-tile t).
- On-device dequant is one fused DVE op per (chunk, t):
  wt[:, t, h, n] = (q_img - 7.5) * s_img (s broadcast over h via a
  step-0 AP), writing straight into the double-buffered weight slab.
  Centering q halves |w| and hence its bf16 rounding error.
- zeros/bias are folded into a rank-65 bf16 matmul accumulated into the
  same PSUM tiles: out += A.T @ C with A[g, m] = group sums of bf16(x)
  plus a ones row (host-computed), C[g, n] = ((7.5 - z)*s).T plus the
  bias row. Using bf16(x) for A makes the x-rounding error cancel
  group-wise against the z-part of the weight.

Measured on trn2.8x1: ~492-497 us HW exec (max over the 8 cores),
relative error ~2.6e-3, TensorE ~86% MFU with the matmul stream pacing
at the 216 ns/MM bf16 roofline.
"""

import numpy as np
import ml_dtypes

import concourse.tile as tile
from concourse import bacc, mybir
from concourse import bass_utils

P = 128
K = 4096
K2 = K // 2
N = 4096
G = 64               # number of groups (K // 64)
GROUPSIZE = 64
N_CORES = 8
M_TOTAL = 8192
M_C = M_TOTAL // N_CORES   # rows per core

NT_CHUNK = 512       # matmul free dim (n per PSUM tile)
KT = K // P          # 32 k'-tiles
TT = KT // 2         # 16 byte-column tiles (nibble halves share scales)
NTC = N // NT_CHUNK  # n-chunks

F32 = mybir.dt.float32
BF16 = mybir.dt.bfloat16
U8 = mybir.dt.uint8

BF16_NP = np.dtype(ml_dtypes.bfloat16)


def build_nc(m_c=M_C, n=N, num_devices=N_CORES, corr_dt=mybir.dt.bfloat16):
    """Build the per-core Bass program (SPMD, no collectives)."""
    mt = m_c // P
    ntc = n // NT_CHUNK

    nc = bacc.Bacc("TRN2", target_bir_lowering=False, debug=False,
                   num_devices=num_devices)

    # x image: [m-tile, partition, k'-tile, m] bf16
    x_d = nc.dram_tensor("x", [mt, P, KT, P], BF16,
                         kind="ExternalInput").ap()
    # q image: [chunk, partition, t, half, n-slice] uint8
    q_d = nc.dram_tensor("q", [ntc, P, TT, 2, NT_CHUNK], U8,
                         kind="ExternalInput").ap()
    # scale image: [chunk, partition, t, n-slice] fp16 (10-bit mantissa
    # keeps the scale rounding negligible; s in [0.001, 0.021])
    s_d = nc.dram_tensor("s", [ntc, P, TT, NT_CHUNK], mybir.dt.float16,
                         kind="ExternalInput").ap()
    # correction rows 0..63 = -(z*s).T, row 64 = bias  -> [65, n]
    c_d = nc.dram_tensor("c", [G + 1, n], corr_dt, kind="ExternalInput").ap()
    # A rows 0..63 = bf16(x) group sums (transposed), row 64 = ones
    a_d = nc.dram_tensor("a", [G + 1, m_c], corr_dt,
                         kind="ExternalInput").ap()
    out_d = nc.dram_tensor("out", [m_c, n], F32, kind="ExternalOutput").ap()

    with tile.TileContext(nc) as tc:
        with (
            tc.tile_pool(name="const", bufs=1) as constp,
            tc.tile_pool(name="xtp", bufs=1) as xtp_pool,
            tc.tile_pool(name="qim", bufs=2) as qpool,
            tc.tile_pool(name="sim", bufs=2) as spool,
            tc.tile_pool(name="wt", bufs=2) as wtp,
            tc.tile_pool(name="cs", bufs=2) as cpool,
            tc.tile_pool(name="outs", bufs=3) as outp,
            tc.tile_pool(name="ps_out", bufs=6, space="PSUM") as ps_out,
            tc.tile_pool(name="ps_warm", bufs=1, space="PSUM") as ps_warm,
        ):
            a_sb = constp.tile([G + 1, m_c], corr_dt)

            xtp = xtp_pool.tile([P, mt, KT, P], BF16)

            def prep_x(mi):
                # x goes on the Activation HWDGE queue so the startup
                # loads run in parallel with the q/s loads on sync
                nc.scalar.dma_start(xtp[:, mi], x_d[mi])

            wts = [None] * ntc
            css = [None] * ntc

            def emit_chunk_inputs(ci):
                """Loads + dequant multiplies for one 512-wide n-chunk."""
                cs = cpool.tile([G + 1, NT_CHUNK], corr_dt, tag="cs",
                                name="cs")
                nc.sync.dma_start(
                    cs[:], c_d[:, ci * NT_CHUNK:(ci + 1) * NT_CHUNK])
                css[ci] = cs
                qim = qpool.tile([P, TT, 2, NT_CHUNK], U8, tag="qim",
                                 name="qim")
                sim = spool.tile([P, TT, NT_CHUNK], mybir.dt.float16,
                                 tag="sim", name="sim")
                # load in slices so the first dequant multiply (and hence
                # the first matmul of the chunk) starts earlier; finest for
                # chunk 0 which gates kernel startup
                step = 2 if ci == 0 else 4
                for tq in range(0, TT, step):
                    nc.sync.dma_start(
                        qim[:, tq:tq + step], q_d[ci, :, tq:tq + step])
                    nc.sync.dma_start(
                        sim[:, tq:tq + step], s_d[ci, :, tq:tq + step])
                wt = wtp.tile([P, TT, 2, NT_CHUNK], BF16, tag="wt",
                              name="wt")
                wts[ci] = wt
                for t in range(TT):
                    s_bc = sim[:, t, None, :].broadcast_to((P, 2, NT_CHUNK))
                    # centered dequant: w = (q - 7.5) * s. Halving the
                    # weight magnitude halves its bf16 rounding error; the
                    # 7.5*s mean moves into the rank-65 correction.
                    nc.vector.scalar_tensor_tensor(
                        out=wt[:, t],
                        in0=qim[:, t],
                        scalar=-7.5,
                        in1=s_bc,
                        op0=mybir.AluOpType.add,
                        op1=mybir.AluOpType.mult,
                    )

            # ---- main loop over n-chunks ----
            prep_x(0)
            emit_chunk_inputs(0)
            nc.sync.dma_start(a_sb[:], a_d[:])
            # HAM warmup: ~7us of dummy matmuls chained on the first cs
            # load so they run during the input DMAs and the PE enters the
            # main loop already at K=8/8 (2.4 GHz)
            wps = ps_warm.tile([P, NT_CHUNK], F32, name="wps")
            for _ in range(16):
                nc.tensor.matmul(
                    wps[:], css[0][:, 0:P], css[0][:], start=True, stop=True)
            for mi in range(1, mt):
                prep_x(mi)

            for ntc_i in range(ntc):
                wt = wts[ntc_i]
                for mi in range(mt):
                    pso = ps_out.tile([P, NT_CHUNK], F32, name="pso")
                    for kt_i in range(KT):
                        # k'-tile kt_i = t + 16h lives at wt[:, t, h, :]
                        h, t = divmod(kt_i, TT)
                        nc.tensor.matmul(
                            pso[:],
                            xtp[:, mi, kt_i],
                            wt[:, t, h],
                            start=(kt_i == 0),
                            stop=False,
                        )
                    # zeros/bias correction: out += A.T @ C
                    nc.tensor.matmul(
                        pso[:],
                        a_sb[:, mi * P:(mi + 1) * P],
                        css[ntc_i][:],
                        start=False,
                        stop=True,
                    )
                    if ntc_i + 1 < ntc and mi == 0:
                        emit_chunk_inputs(ntc_i + 1)
                    ot = outp.tile([P, NT_CHUNK], F32, name="ot")
                    nc.scalar.copy(ot[:], pso[:])
                    nc.scalar.dma_start(
                        out_d[mi * P:(mi + 1) * P,
                              ntc_i * NT_CHUNK:(ntc_i + 1) * NT_CHUNK],
                        ot[:],
                    )

    nc.compile()
    return nc


def prep_inputs(x, packed_weight, scales, zeros, bias):
    """Host-side input preparation -> per-core input maps."""
    xf = x.reshape(M_TOTAL, K)
    # bf16, k'-permuted (even | odd), transposed, tiled to the SBUF image
    xt_bf = np.empty((K, M_TOTAL), dtype=BF16_NP)
    xt_bf[:K2] = xf[:, 0::2].astype(BF16_NP).T
    xt_bf[K2:] = xf[:, 1::2].astype(BF16_NP).T

    pwu = packed_weight.astype(np.uint8)            # values are 0..255
    # q image: [chunk, p, t, half, n-slice]; k' = (t + 16h)*128 + p maps to
    # byte column t*128+p, low nibble for h=0, high nibble for h=1
    b = pwu.T.reshape(TT, P, N)                      # [t, p, n]
    q_img = np.stack([b & 15, b >> 4], axis=2)       # [t, p, 2, n]
    q_img = np.ascontiguousarray(
        q_img.reshape(TT, P, 2, NTC, NT_CHUNK)
        .transpose(3, 1, 0, 2, 4))                   # [chunk, p, t, 2, ns]

    # s image: [chunk, p, t, n-slice] with s_img[p, t, n] = s[n, 4t + p//32]
    sT = scales.astype(np.float32).T                 # [64, N]
    s_img = np.repeat(sT.reshape(TT, 4, 1, N), 32, axis=2)  # [t, 4, 32, n]
    s_img = np.ascontiguousarray(
        s_img.reshape(TT, P, NTC, NT_CHUNK)
        .transpose(2, 1, 0, 3)).astype(np.float16)   # [chunk, p, t, ns]

    c_host = np.concatenate(
        [((7.5 - zeros) * scales).T, bias[None, :]],
        axis=0).astype(BF16_NP)
    # A: per-group sums of bf16(x) plus ones row (bf16 so the x-rounding
    # error cancels against the z-part of the dequantized weight)
    a_full = xf.astype(BF16_NP).astype(np.float32).reshape(
        M_TOTAL, G, GROUPSIZE).sum(axis=2)           # [M, 64]

    in_maps = []
    for c in range(N_CORES):
        sl = slice(c * M_C, (c + 1) * M_C)
        x_img = np.ascontiguousarray(
            xt_bf[:, sl].reshape(KT, P, M_C // P, P).transpose(2, 1, 0, 3))
        a_slab = np.concatenate(
            [a_full[sl].T, np.ones((1, M_C), np.float32)],
            axis=0).astype(BF16_NP)
        in_maps.append({
            "x": x_img,
            "q": q_img,
            "s": s_img,
            "c": c_host,
            "a": np.ascontiguousarray(a_slab),
        })
    return in_maps


_NC_CACHE = {}


def get_nc():
    if "nc" not in _NC_CACHE:
        _NC_CACHE["nc"] = build_nc()
    return _NC_CACHE["nc"]


def kernel(x, packed_weight, scales, zeros, bias):
    x = np.asarray(x)
    packed_weight = np.asarray(packed_weight)
    scales = np.asarray(scales, dtype=np.float32)
    zeros = np.asarray(zeros, dtype=np.float32)
    bias = np.asarray(bias, dtype=np.float32)
    nc = get_nc()
    in_maps = prep_inputs(x, packed_weight, scales, zeros, bias)
    res = bass_utils.run_bass_kernel_spmd(
        nc, in_maps, core_ids=list(range(N_CORES)))
    out = np.concatenate([r["out"] for r in res.results], axis=0)
    return out.reshape(*x.shape[:-1], N).astype(np.float32)

